# revision 3
# baseline (speedup 1.0000x reference)
"""TRN2 Bass kernel for nn_Encoder v2: 2-layer LSTM, batch-sharded over 8 cores.

Per core (B=2048): 4 batch groups x 512, row = 32g+16*layer+j, L2 one step
behind L1. v2 changes vs baseline: x-projection merged into ONE block-diagonal
bf16 matmul per gate plane (4 small per-group matmuls -> 1), so each plane is
3 matmuls (wq-BD start, wr f32r, wh f32r stop). x is staged in DRAM as
xr5[512, T, 4, 32] bf16 (batch-in-group major, (g,feat) interleaved last) via
slab-pipelined load -> gpsimd-cast -> store, then ONE contiguous DMA-transpose
per timestep yields the [128=(4g x 32k), 512] rhs tile.
"""
import sys
sys.path.insert(0, '/opt/trn_rl_repo')
import numpy as np

B, T, F, H = 2048, 100, 18, 16
G = 4
BG = B // G          # 512 batch per group
NS = 2               # column halves
CW = BG // NS        # 256
N_CORES = 8
SLAB = 10            # timesteps per staging slab
NSLAB = T // SLAB
PLANE_SLICE = [16, 0, 32, 48]   # PSUM plane X -> weight row block (f,i,g,o)
PLANE_SCALE = [1.0, 1.0, 2.0, 1.0]  # g-plane x2: tanh(x) = 2*sigmoid(2x)-1

_cache = {}


def _pack_weights(W_ih1, W_hh1, b_ih1, b_hh1, W_ih2, W_hh2, b_ih2, b_hh2):
    import ml_dtypes
    wq = np.zeros((4, 128, 128), np.float32)   # block-diag x-proj lhsT (-> bf16)
    wl = np.zeros((4, 128, 128), np.float32)   # bias-only lhsT for final round
    wr = np.zeros((4, 128, 128), np.float32)   # L2-input proj from relu(h)
    wh = np.zeros((4, 128, 128), np.float32)   # recurrence
    for X in range(4):
        s0 = PLANE_SLICE[X]
        sc = PLANE_SCALE[X]
        b1 = sc * (b_ih1[s0:s0 + 16] + b_hh1[s0:s0 + 16])
        b2 = sc * (b_ih2[s0:s0 + 16] + b_hh2[s0:s0 + 16])
        b1a = b1.astype(ml_dtypes.bfloat16).astype(np.float32)
        b2a = b2.astype(ml_dtypes.bfloat16).astype(np.float32)
        for g in range(G):
            r0 = 32 * g
            wq[X, r0:r0 + F, r0:r0 + 16] = sc * W_ih1[s0:s0 + 16, :].T
            wq[X, r0 + F, r0:r0 + 16] = b1a
            wq[X, r0 + F, r0 + 16:r0 + 32] = b2a
            wq[X, r0 + F + 1, r0:r0 + 16] = b1 - b1a
            wq[X, r0 + F + 1, r0 + 16:r0 + 32] = b2 - b2a
            wl[X, r0 + F, r0:r0 + 16] = b1a
            wl[X, r0 + F, r0 + 16:r0 + 32] = b2a
            wl[X, r0 + F + 1, r0:r0 + 16] = b1 - b1a
            wl[X, r0 + F + 1, r0 + 16:r0 + 32] = b2 - b2a
            wr[X, r0:r0 + 16, r0 + 16:r0 + 32] = sc * W_ih2[s0:s0 + 16, :].T
            wh[X, r0:r0 + 16, r0:r0 + 16] = sc * W_hh1[s0:s0 + 16, :].T
            wh[X, r0 + 16:r0 + 32, r0 + 16:r0 + 32] = sc * W_hh2[s0:s0 + 16, :].T
    mk = np.ones((128, 1), np.float32)          # L1 mask: zero the L2 rows
    for g in range(G):
        mk[32 * g + 16:32 * g + 32] = 0.0
    return wq, wr, wh, wl, mk


def _build():
    import concourse.bacc as bacc
    import concourse.tile as tile
    from concourse import mybir
    from concourse.masks import make_identity
    f32, f32r, bf16 = mybir.dt.float32, mybir.dt.float32r, mybir.dt.bfloat16
    AF, ALU = mybir.ActivationFunctionType, mybir.AluOpType
    R = T
    nc = bacc.Bacc(None, target_bir_lowering=False)
    xt_d = nc.dram_tensor("xt", [T, 128, BG], bf16, kind="ExternalInput")
    wq_d = nc.dram_tensor("wq", [4, 128, 128], f32, kind="ExternalInput")
    wr_d = nc.dram_tensor("wr", [4, 128, 128], f32, kind="ExternalInput")
    wh_d = nc.dram_tensor("wh", [4, 128, 128], f32, kind="ExternalInput")
    wl_d = nc.dram_tensor("wl", [4, 128, 128], f32, kind="ExternalInput")
    mk_d = nc.dram_tensor("mk", [128, 1], f32, kind="ExternalInput")
    y_d = nc.dram_tensor("y", [B, H], f32, kind="ExternalOutput")

    with tile.TileContext(nc) as tc:
        with tc.tile_pool(name="singles", bufs=1) as singles, \
             tc.tile_pool(name="xt", bufs=2) as xtp, \
             tc.tile_pool(name="ys", bufs=4) as ysp, \
             tc.tile_pool(name="ps", bufs=4, space="PSUM") as psp:

            # ---- weights / constants. wq loads first: round 0 needs only
            # it (r1=h=0 there, so the wr/wh matmuls are skipped). ----
            def load_w(d, tag):
                wf = singles.tile([128, 4, 128], f32, tag=f"{tag}_f")
                nc.sync.dma_start(out=wf, in_=d.rearrange("x k m -> k x m"))
                w = singles.tile([128, 4, 128], bf16, tag=tag)
                nc.vector.tensor_copy(out=w, in_=wf)
                return w
            wq = load_w(wq_d, "wq")
            mk = singles.tile([128, 1], f32)
            nc.sync.dma_start(out=mk, in_=mk_d[:, :])
            ident = singles.tile([128, 128], f32)
            make_identity(nc, ident)

            # ---- states (per column-half) ----
            h_h, c_h, tg_h, r1_h, s_h, tp0_h, tp1_h, tn_h = [], [], [], [], [], [], [], []
            for i in range(NS):
                h_i = singles.tile([128, CW], bf16, tag=f"h{i}")
                nc.vector.memset(h_i, 0.0)
                h_h.append(h_i)
                c_i = singles.tile([128, CW], bf16, tag=f"c{i}")      # cell state
                nc.vector.memset(c_i, 0.0)
                c_h.append(c_i)
                tg_i = singles.tile([128, CW], bf16, tag=f"tg{i}")    # tanh(G)
                tg_h.append(tg_i)
                r1_i = singles.tile([128, CW], bf16, tag=f"r1{i}")    # relu(h)
                nc.vector.memset(r1_i, 0.0)
                r1_h.append(r1_i)
                s_i = singles.tile([128, 4, CW], bf16, tag=f"s{i}")   # sF sI sG2 sO
                s_h.append(s_i)
                tp0_i = singles.tile([128, CW], bf16, tag=f"tp0{i}")  # f*c
                tp0_h.append(tp0_i)
                tp1_i = singles.tile([128, CW], bf16, tag=f"tp1{i}")  # i*g
                tp1_h.append(tp1_i)
                tn_i = singles.tile([128, CW], bf16, tag=f"tn{i}")    # tanh(c)
                tn_h.append(tn_i)

            xts = {}

            def emit_xt(t):
                xt = xtp.tile([128, BG], bf16, tag=f"xt{t % 4}")
                nc.sync.dma_start(out=xt, in_=xt_d[t, :, :])
                xts[t] = xt

            emit_xt(0)
            emit_xt(1)
            emit_xt(2)
            next_xt = 3
            wr = load_w(wr_d, "wr")
            wh = load_w(wh_d, "wh")
            wl = load_w(wl_d, "wl")

            for r in range(R + 1):
                l1 = r < R
                if l1:
                    while next_xt <= min(r + 4, R - 1):
                        emit_xt(next_xt)
                        next_xt += 1
                    xt_cur, wsel = xts[r], wq
                else:
                    xt_cur, wsel = xts[R - 1], wl
                # PSUM planes: F=0, I=1, O=2, G=3. Phase-interleave the two
                # column halves so the in-order Act engine never head-of-line
                # blocks: tgA sigA tgB sigB tnA tnB.
                Ps = []
                for hf in range(NS):
                    cs = slice(CW * hf, CW * (hf + 1))
                    P = psp.tile([128, 4, CW], f32)
                    Ps.append(P)
                    # One open PSUM accumulation group per bank (planes
                    # {0,1}=F,I and {2,3}=G,O share banks). wq_F/wq_G have no
                    # state dep and pre-run while PE waits on r1; F and G
                    # chains close before I opens so sigmoid(F,I,G) fires at
                    # the 9th matmul.
                    def mm(X, w, rhs, st, sp):
                        nc.tensor.matmul(P[:, X, :], w[:, X, :], rhs,
                                         start=st, stop=sp, skip_group_check=True)
                    if r == 0:
                        # round 0: r1 = h = 0, so gates are x-proj + bias only
                        for X in (0, 1, 2, 3):
                            mm(X, wsel, xt_cur[:, cs], True, True)
                    else:
                        mm(0, wsel, xt_cur[:, cs], True, False)   # wq_F
                        mm(2, wsel, xt_cur[:, cs], True, False)   # wq_G
                        mm(0, wr, r1_h[hf], False, False)
                        mm(0, wh, h_h[hf], False, True)           # F stop
                        mm(2, wr, r1_h[hf], False, False)
                        mm(2, wh, h_h[hf], False, True)           # G stop
                        mm(1, wsel, xt_cur[:, cs], True, False)   # wq_I
                        mm(1, wr, r1_h[hf], False, False)
                        mm(1, wh, h_h[hf], False, True)           # I stop
                        mm(3, wsel, xt_cur[:, cs], True, False)   # wq_O
                        mm(3, wr, r1_h[hf], False, False)
                        mm(3, wh, h_h[hf], False, True)           # O stop
                for hf in range(NS):
                    # One fused sigmoid over F,I,G2 planes (g weights carry x2:
                    # tanh(x) = 2*sigmoid(2x)-1 reconstructed on DVE below).
                    nc.scalar.activation(out=s_h[hf][:, 0:3, :], in_=Ps[hf][:, 0:3, :],
                                         func=AF.Sigmoid)
                    nc.scalar.activation(out=s_h[hf][:, 3, :], in_=Ps[hf][:, 3, :],
                                         func=AF.Sigmoid)
                for hf in range(NS):
                    nc.vector.tensor_tensor(out=tp0_h[hf], in0=s_h[hf][:, 0, :],
                                            in1=c_h[hf], op=ALU.mult)
                    nc.vector.tensor_scalar(out=tg_h[hf], in0=s_h[hf][:, 2, :],
                                            scalar1=2.0, scalar2=-1.0,
                                            op0=ALU.mult, op1=ALU.add)
                    nc.vector.tensor_tensor(out=tp1_h[hf], in0=s_h[hf][:, 1, :],
                                            in1=tg_h[hf], op=ALU.mult)
                    nc.vector.tensor_tensor(out=c_h[hf], in0=tp0_h[hf],
                                            in1=tp1_h[hf], op=ALU.add)
                for hf in range(NS):
                    nc.scalar.activation(out=tn_h[hf], in_=c_h[hf], func=AF.Tanh)
                for hf in range(NS):
                    # r1 = relu(h) = max(tanh(c),0)*sig(O) since sig(O)>0.
                    nc.vector.scalar_tensor_tensor(
                        out=r1_h[hf], in0=tn_h[hf], scalar=0.0, in1=s_h[hf][:, 3, :],
                        op0=ALU.max, op1=ALU.mult)
                    if l1:
                        nc.vector.tensor_tensor(out=h_h[hf], in0=s_h[hf][:, 3, :],
                                                in1=tn_h[hf], op=ALU.mult)
                if r == 0:
                    # L2 starts at r=1 from zero state: mask-wipe its rows
                    for hf in range(NS):
                        nc.vector.tensor_scalar(out=h_h[hf], in0=h_h[hf], scalar1=mk,
                                                scalar2=None, op0=ALU.mult)
                        nc.vector.tensor_scalar(out=c_h[hf], in0=c_h[hf],
                                                scalar1=mk, scalar2=None, op0=ALU.mult)

            # ---- output: r1 == relu(h); widen to f32, transpose out ----
            yrf = singles.tile([128, BG], f32)
            for hf in range(NS):
                nc.vector.tensor_copy(out=yrf[:, CW * hf:CW * (hf + 1)],
                                      in_=r1_h[hf])
            yv = y_d.rearrange("(g b) h -> b g h", g=G)  # [512, 4, 16]
            for j in range(4):
                P = psp.tile([128, 4, CW], f32)
                po = P[:, 0, 0:128]
                nc.tensor.transpose(po, yrf[:, 128 * j:128 * (j + 1)], ident)
                ys = ysp.tile([128, 128], f32, tag="ys")
                nc.vector.tensor_copy(out=ys, in_=po)
                nc.sync.dma_start(
                    out=yv[128 * j:128 * (j + 1), :, :],
                    in_=ys[:, :].rearrange("p (g w) -> p g w", w=32)[:, :, 16:32])
    nc.finalize()
    return nc


def _pack_x(x):
    """x [B*8, T, F] f32 -> per-core [T, 128, BG] bf16 tiles: row 32g+k holds
    feature k of batch-group g (k=18,19 are const 1.0 bias lanes)."""
    import ml_dtypes
    nb = x.shape[0] // B
    xp = np.zeros((nb, T, G, 32, BG), np.float32)
    xr = x.reshape(nb, G, BG, T, F)
    xp[:, :, :, 0:F, :] = xr.transpose(0, 3, 1, 4, 2)
    xp[:, :, :, F:F + 2, :] = 1.0
    return np.ascontiguousarray(
        xp.reshape(nb, T, 128, BG)).astype(ml_dtypes.bfloat16)


def kernel(x, W_ih1, W_hh1, b_ih1, b_hh1, W_ih2, W_hh2, b_ih2, b_hh2):
    from concourse.bass_utils import run_bass_kernel_spmd
    if "nc" not in _cache:
        _cache["nc"] = _build()
    nc = _cache["nc"]
    wq, wr, wh, wl, mk = _pack_weights(
        np.asarray(W_ih1), np.asarray(W_hh1), np.asarray(b_ih1), np.asarray(b_hh1),
        np.asarray(W_ih2), np.asarray(W_hh2), np.asarray(b_ih2), np.asarray(b_hh2))
    x = np.ascontiguousarray(np.asarray(x), dtype=np.float32)
    xp = _pack_x(x)
    in_maps = [dict(xt=xp[i], wq=wq, wr=wr, wh=wh, wl=wl, mk=mk)
               for i in range(N_CORES)]
    res = run_bass_kernel_spmd(nc, in_maps, core_ids=list(range(N_CORES)))
    return np.concatenate([res.results[i]["y"] for i in range(N_CORES)], axis=0)


# revision 5
# speedup vs baseline: 1.0023x; 1.0023x over previous
"""TRN2 Bass kernel for nn_Encoder: 2-layer LSTM encoder, batch-sharded 8 ways.

Per core (B=2048): 4 batch groups x 512 batch columns; partition row =
32g + 16*layer + j; L2 runs one timestep behind L1 in the same tiles, so one
set of matmuls serves both layers. Columns split in 2 halves (CW=256) that
pipeline against each other.

Per round and half, each gate plane is 3 bf16 matmuls accumulated in PSUM:
block-diagonal x-projection (lhsT has per-group W_ih1 blocks + bias rows fed
by constant-1 lanes of x), wr (relu(h1) -> L2 inputs), wh (recurrence); PSUM
allows one open accumulation group per bank, so plane chains close before a
same-bank plane starts, with the two x-projections of bank-distinct planes
pre-issued while PE waits on the recurrence.

Elementwise: the g-gate weights carry x2 so one fused sigmoid covers F/I/G
planes (tanh(x) = 2*sigmoid(2x)-1, reconstructed by a 4x-mode tensor_scalar);
sigma(O) separate; cell state kept in bf16; relu(h) computed directly as
max(tanh(c),0)*sigma(O) via scalar_tensor_tensor; h on GPSIMD. Host-side
packing pre-transposes x into per-timestep [128, 512] bf16 tiles (one plain
DMA per round) and packs the weight/bias matrices.
"""
import sys
sys.path.insert(0, '/opt/trn_rl_repo')
import numpy as np

B, T, F, H = 2048, 100, 18, 16
G = 4
BG = B // G          # 512 batch per group
NS = 2               # column halves
CW = BG // NS        # 256
N_CORES = 8
SLAB = 10            # timesteps per staging slab
NSLAB = T // SLAB
PLANE_SLICE = [16, 0, 32, 48]   # PSUM plane X -> weight row block (f,i,g,o)
PLANE_SCALE = [1.0, 1.0, 2.0, 1.0]  # g-plane x2: tanh(x) = 2*sigmoid(2x)-1

_cache = {}


def _pack_weights(W_ih1, W_hh1, b_ih1, b_hh1, W_ih2, W_hh2, b_ih2, b_hh2):
    import ml_dtypes
    wq = np.zeros((4, 128, 128), np.float32)   # block-diag x-proj lhsT (-> bf16)
    wl = np.zeros((4, 128, 128), np.float32)   # bias-only lhsT for final round
    wr = np.zeros((4, 128, 128), np.float32)   # L2-input proj from relu(h)
    wh = np.zeros((4, 128, 128), np.float32)   # recurrence
    for X in range(4):
        s0 = PLANE_SLICE[X]
        sc = PLANE_SCALE[X]
        b1 = sc * (b_ih1[s0:s0 + 16] + b_hh1[s0:s0 + 16])
        b2 = sc * (b_ih2[s0:s0 + 16] + b_hh2[s0:s0 + 16])
        b1a = b1.astype(ml_dtypes.bfloat16).astype(np.float32)
        b2a = b2.astype(ml_dtypes.bfloat16).astype(np.float32)
        for g in range(G):
            r0 = 32 * g
            wq[X, r0:r0 + F, r0:r0 + 16] = sc * W_ih1[s0:s0 + 16, :].T
            wq[X, r0 + F, r0:r0 + 16] = b1a
            wq[X, r0 + F, r0 + 16:r0 + 32] = b2a
            wq[X, r0 + F + 1, r0:r0 + 16] = b1 - b1a
            wq[X, r0 + F + 1, r0 + 16:r0 + 32] = b2 - b2a
            wl[X, r0 + F, r0:r0 + 16] = b1a
            wl[X, r0 + F, r0 + 16:r0 + 32] = b2a
            wl[X, r0 + F + 1, r0:r0 + 16] = b1 - b1a
            wl[X, r0 + F + 1, r0 + 16:r0 + 32] = b2 - b2a
            wr[X, r0:r0 + 16, r0 + 16:r0 + 32] = sc * W_ih2[s0:s0 + 16, :].T
            wh[X, r0:r0 + 16, r0:r0 + 16] = sc * W_hh1[s0:s0 + 16, :].T
            wh[X, r0 + 16:r0 + 32, r0 + 16:r0 + 32] = sc * W_hh2[s0:s0 + 16, :].T
    mk = np.ones((128, 1), np.float32)          # L1 mask: zero the L2 rows
    for g in range(G):
        mk[32 * g + 16:32 * g + 32] = 0.0
    return wq, wr, wh, wl, mk


def _build():
    import concourse.bacc as bacc
    import concourse.tile as tile
    from concourse import mybir
    from concourse.masks import make_identity
    f32, f32r, bf16 = mybir.dt.float32, mybir.dt.float32r, mybir.dt.bfloat16
    AF, ALU = mybir.ActivationFunctionType, mybir.AluOpType
    R = T
    nc = bacc.Bacc(None, target_bir_lowering=False)
    xt_d = nc.dram_tensor("xt", [T, 128, BG], bf16, kind="ExternalInput")
    wq_d = nc.dram_tensor("wq", [4, 128, 128], f32, kind="ExternalInput")
    wr_d = nc.dram_tensor("wr", [4, 128, 128], f32, kind="ExternalInput")
    wh_d = nc.dram_tensor("wh", [4, 128, 128], f32, kind="ExternalInput")
    wl_d = nc.dram_tensor("wl", [4, 128, 128], f32, kind="ExternalInput")
    mk_d = nc.dram_tensor("mk", [128, 1], f32, kind="ExternalInput")
    y_d = nc.dram_tensor("y", [B, H], f32, kind="ExternalOutput")

    with tile.TileContext(nc) as tc:
        with tc.tile_pool(name="singles", bufs=1) as singles, \
             tc.tile_pool(name="xt", bufs=2) as xtp, \
             tc.tile_pool(name="ys", bufs=4) as ysp, \
             tc.tile_pool(name="ps", bufs=4, space="PSUM") as psp:

            # ---- weights / constants. wq loads first: round 0 needs only
            # it (r1=h=0 there, so the wr/wh matmuls are skipped). ----
            def load_w(d, tag):
                wf = singles.tile([128, 4, 128], f32, tag=f"{tag}_f")
                nc.sync.dma_start(out=wf, in_=d.rearrange("x k m -> k x m"))
                w = singles.tile([128, 4, 128], bf16, tag=tag)
                nc.vector.tensor_copy(out=w, in_=wf)
                return w
            wq = load_w(wq_d, "wq")
            mk = singles.tile([128, 1], f32)
            nc.sync.dma_start(out=mk, in_=mk_d[:, :])
            ident = singles.tile([128, 128], f32)
            make_identity(nc, ident)

            # ---- states (per column-half) ----
            h_h, c_h, tg_h, r1_h, s_h, tp0_h, tp1_h, tn_h = [], [], [], [], [], [], [], []
            for i in range(NS):
                h_i = singles.tile([128, CW], bf16, tag=f"h{i}")
                nc.vector.memset(h_i, 0.0)
                h_h.append(h_i)
                c_i = singles.tile([128, CW], bf16, tag=f"c{i}")      # cell state
                nc.vector.memset(c_i, 0.0)
                c_h.append(c_i)
                tg_i = singles.tile([128, CW], bf16, tag=f"tg{i}")    # tanh(G)
                tg_h.append(tg_i)
                r1_i = singles.tile([128, CW], bf16, tag=f"r1{i}")    # relu(h)
                nc.vector.memset(r1_i, 0.0)
                r1_h.append(r1_i)
                s_i = singles.tile([128, 4, CW], bf16, tag=f"s{i}")   # sF sI sG2 sO
                s_h.append(s_i)
                tp0_i = singles.tile([128, CW], bf16, tag=f"tp0{i}")  # f*c
                tp0_h.append(tp0_i)
                tp1_i = singles.tile([128, CW], bf16, tag=f"tp1{i}")  # i*g
                tp1_h.append(tp1_i)
                tn_i = singles.tile([128, CW], bf16, tag=f"tn{i}")    # tanh(c)
                tn_h.append(tn_i)

            xts = {}

            def emit_xt(t):
                xt = xtp.tile([128, BG], bf16, tag=f"xt{t % 4}")
                nc.sync.dma_start(out=xt, in_=xt_d[t, :, :])
                xts[t] = xt

            emit_xt(0)
            emit_xt(1)
            emit_xt(2)
            next_xt = 3
            wr = load_w(wr_d, "wr")
            wh = load_w(wh_d, "wh")
            wl = load_w(wl_d, "wl")

            for r in range(R + 1):
                l1 = r < R
                if l1:
                    while next_xt <= min(r + 4, R - 1):
                        emit_xt(next_xt)
                        next_xt += 1
                    xt_cur, wsel = xts[r], wq
                else:
                    xt_cur, wsel = xts[R - 1], wl
                # PSUM planes: F=0, I=1, O=2, G=3. Phase-interleave the two
                # column halves so the in-order Act engine never head-of-line
                # blocks: tgA sigA tgB sigB tnA tnB.
                Ps = []
                for hf in range(NS):
                    cs = slice(CW * hf, CW * (hf + 1))
                    P = psp.tile([128, 4, CW], f32)
                    Ps.append(P)
                    # One open PSUM accumulation group per bank (planes
                    # {0,1}=F,I and {2,3}=G,O share banks). wq_F/wq_G have no
                    # state dep and pre-run while PE waits on r1; F and G
                    # chains close before I opens so sigmoid(F,I,G) fires at
                    # the 9th matmul.
                    def mm(X, w, rhs, st, sp):
                        nc.tensor.matmul(P[:, X, :], w[:, X, :], rhs,
                                         start=st, stop=sp, skip_group_check=True)
                    if r == 0:
                        # round 0: r1 = h = 0, so gates are x-proj + bias only
                        for X in (0, 1, 2, 3):
                            mm(X, wsel, xt_cur[:, cs], True, True)
                    else:
                        mm(0, wsel, xt_cur[:, cs], True, False)   # wq_F
                        mm(2, wsel, xt_cur[:, cs], True, False)   # wq_G
                        mm(0, wr, r1_h[hf], False, False)
                        mm(0, wh, h_h[hf], False, True)           # F stop
                        mm(2, wr, r1_h[hf], False, False)
                        mm(2, wh, h_h[hf], False, True)           # G stop
                        mm(1, wsel, xt_cur[:, cs], True, False)   # wq_I
                        mm(1, wr, r1_h[hf], False, False)
                        mm(1, wh, h_h[hf], False, True)           # I stop
                        mm(3, wsel, xt_cur[:, cs], True, False)   # wq_O
                        mm(3, wr, r1_h[hf], False, False)
                        mm(3, wh, h_h[hf], False, True)           # O stop
                for hf in range(NS):
                    # One fused sigmoid over F,I,G2 planes (g weights carry x2:
                    # tanh(x) = 2*sigmoid(2x)-1 reconstructed on DVE below).
                    nc.scalar.activation(out=s_h[hf][:, 0:3, :], in_=Ps[hf][:, 0:3, :],
                                         func=AF.Sigmoid)
                    nc.scalar.activation(out=s_h[hf][:, 3, :], in_=Ps[hf][:, 3, :],
                                         func=AF.Sigmoid)
                for hf in range(NS):
                    nc.vector.tensor_tensor(out=tp0_h[hf], in0=s_h[hf][:, 0, :],
                                            in1=c_h[hf], op=ALU.mult)
                    nc.vector.tensor_scalar(out=tg_h[hf], in0=s_h[hf][:, 2, :],
                                            scalar1=2.0, scalar2=-1.0,
                                            op0=ALU.mult, op1=ALU.add)
                    nc.vector.tensor_tensor(out=tp1_h[hf], in0=s_h[hf][:, 1, :],
                                            in1=tg_h[hf], op=ALU.mult)
                    nc.vector.tensor_tensor(out=c_h[hf], in0=tp0_h[hf],
                                            in1=tp1_h[hf], op=ALU.add)
                for hf in range(NS):
                    nc.scalar.activation(out=tn_h[hf], in_=c_h[hf], func=AF.Tanh)
                for hf in range(NS):
                    # r1 = relu(h) = max(tanh(c),0)*sig(O) since sig(O)>0.
                    nc.vector.scalar_tensor_tensor(
                        out=r1_h[hf], in0=tn_h[hf], scalar=0.0, in1=s_h[hf][:, 3, :],
                        op0=ALU.max, op1=ALU.mult)
                    if l1:
                        nc.gpsimd.tensor_tensor(out=h_h[hf], in0=s_h[hf][:, 3, :],
                                                in1=tn_h[hf], op=ALU.mult)
                if r == 0:
                    # L2 starts at r=1 from zero state: mask-wipe its rows
                    for hf in range(NS):
                        nc.vector.tensor_scalar(out=h_h[hf], in0=h_h[hf], scalar1=mk,
                                                scalar2=None, op0=ALU.mult)
                        nc.vector.tensor_scalar(out=c_h[hf], in0=c_h[hf],
                                                scalar1=mk, scalar2=None, op0=ALU.mult)

            # ---- output: r1 == relu(h); widen to f32, transpose out ----
            yrf = singles.tile([128, BG], f32)
            for hf in range(NS):
                nc.vector.tensor_copy(out=yrf[:, CW * hf:CW * (hf + 1)],
                                      in_=r1_h[hf])
            yv = y_d.rearrange("(g b) h -> b g h", g=G)  # [512, 4, 16]
            for j in range(4):
                P = psp.tile([128, 4, CW], f32)
                po = P[:, 0, 0:128]
                nc.tensor.transpose(po, yrf[:, 128 * j:128 * (j + 1)], ident)
                ys = ysp.tile([128, 128], f32, tag="ys")
                nc.vector.tensor_copy(out=ys, in_=po)
                nc.sync.dma_start(
                    out=yv[128 * j:128 * (j + 1), :, :],
                    in_=ys[:, :].rearrange("p (g w) -> p g w", w=32)[:, :, 16:32])
    nc.finalize()
    return nc


def _pack_x(x):
    """x [B*8, T, F] f32 -> per-core [T, 128, BG] bf16 tiles: row 32g+k holds
    feature k of batch-group g (k=18,19 are const 1.0 bias lanes)."""
    import ml_dtypes
    nb = x.shape[0] // B
    xp = np.zeros((nb, T, G, 32, BG), np.float32)
    xr = x.reshape(nb, G, BG, T, F)
    xp[:, :, :, 0:F, :] = xr.transpose(0, 3, 1, 4, 2)
    xp[:, :, :, F:F + 2, :] = 1.0
    return np.ascontiguousarray(
        xp.reshape(nb, T, 128, BG)).astype(ml_dtypes.bfloat16)


def kernel(x, W_ih1, W_hh1, b_ih1, b_hh1, W_ih2, W_hh2, b_ih2, b_hh2):
    from concourse.bass_utils import run_bass_kernel_spmd
    if "nc" not in _cache:
        _cache["nc"] = _build()
    nc = _cache["nc"]
    wq, wr, wh, wl, mk = _pack_weights(
        np.asarray(W_ih1), np.asarray(W_hh1), np.asarray(b_ih1), np.asarray(b_hh1),
        np.asarray(W_ih2), np.asarray(W_hh2), np.asarray(b_ih2), np.asarray(b_hh2))
    x = np.ascontiguousarray(np.asarray(x), dtype=np.float32)
    xp = _pack_x(x)
    in_maps = [dict(xt=xp[i], wq=wq, wr=wr, wh=wh, wl=wl, mk=mk)
               for i in range(N_CORES)]
    res = run_bass_kernel_spmd(nc, in_maps, core_ids=list(range(N_CORES)))
    return np.concatenate([res.results[i]["y"] for i in range(N_CORES)], axis=0)


# revision 6
# speedup vs baseline: 1.0243x; 1.0220x over previous
"""TRN2 Bass kernel for nn_Encoder: 2-layer LSTM encoder, batch-sharded 8 ways.

Per core (B=2048): 4 batch groups x 512 batch columns; partition row =
32g + 16*layer + j; L2 runs one timestep behind L1 in the same tiles, so one
set of matmuls serves both layers. Columns split in 2 halves (CW=256) that
pipeline against each other.

Per round and half, each gate plane is 3 bf16 matmuls accumulated in PSUM:
block-diagonal x-projection (lhsT has per-group W_ih1 blocks + bias rows fed
by constant-1 lanes of x), wr (relu(h1) -> L2 inputs), wh (recurrence); PSUM
allows one open accumulation group per bank, so plane chains close before a
same-bank plane starts, with the two x-projections of bank-distinct planes
pre-issued while PE waits on the recurrence.

Elementwise: the g-gate weights carry x2 so one fused sigmoid covers F/I/G
planes (tanh(x) = 2*sigmoid(2x)-1, reconstructed by a 4x-mode tensor_scalar);
sigma(O) separate; cell state kept in bf16; relu(h) computed directly as
max(tanh(c),0)*sigma(O) via scalar_tensor_tensor; h on GPSIMD. Host-side
packing pre-transposes x into per-timestep [128, 512] bf16 tiles (one plain
DMA per round) and packs the weight/bias matrices.
"""
import sys
sys.path.insert(0, '/opt/trn_rl_repo')
import numpy as np

B, T, F, H = 2048, 100, 18, 16
G = 4
BG = B // G          # 512 batch per group
NS = 2               # column halves
CW = BG // NS        # 256
N_CORES = 8
SLAB = 10            # timesteps per staging slab
NSLAB = T // SLAB
PLANE_SLICE = [16, 0, 32, 48]   # PSUM plane X -> weight row block (f,i,g,o)
PLANE_SCALE = [1.0, 1.0, 2.0, 1.0]  # g-plane x2: tanh(x) = 2*sigmoid(2x)-1

_cache = {}


def _pack_weights(W_ih1, W_hh1, b_ih1, b_hh1, W_ih2, W_hh2, b_ih2, b_hh2):
    import ml_dtypes
    wq = np.zeros((4, 128, 128), np.float32)   # block-diag x-proj lhsT (-> bf16)
    wl = np.zeros((4, 128, 128), np.float32)   # bias-only lhsT for final round
    wr = np.zeros((4, 128, 128), np.float32)   # L2-input proj from relu(h)
    wh = np.zeros((4, 128, 128), np.float32)   # recurrence
    for X in range(4):
        s0 = PLANE_SLICE[X]
        sc = PLANE_SCALE[X]
        b1 = sc * (b_ih1[s0:s0 + 16] + b_hh1[s0:s0 + 16])
        b2 = sc * (b_ih2[s0:s0 + 16] + b_hh2[s0:s0 + 16])
        b1a = b1.astype(ml_dtypes.bfloat16).astype(np.float32)
        b2a = b2.astype(ml_dtypes.bfloat16).astype(np.float32)
        for g in range(G):
            r0 = 32 * g
            wq[X, r0:r0 + F, r0:r0 + 16] = sc * W_ih1[s0:s0 + 16, :].T
            wq[X, r0 + F, r0:r0 + 16] = b1a
            wq[X, r0 + F, r0 + 16:r0 + 32] = b2a
            wq[X, r0 + F + 1, r0:r0 + 16] = b1 - b1a
            wq[X, r0 + F + 1, r0 + 16:r0 + 32] = b2 - b2a
            wl[X, r0 + F, r0:r0 + 16] = b1a
            wl[X, r0 + F, r0 + 16:r0 + 32] = b2a
            wl[X, r0 + F + 1, r0:r0 + 16] = b1 - b1a
            wl[X, r0 + F + 1, r0 + 16:r0 + 32] = b2 - b2a
            wr[X, r0:r0 + 16, r0 + 16:r0 + 32] = sc * W_ih2[s0:s0 + 16, :].T
            wh[X, r0:r0 + 16, r0:r0 + 16] = sc * W_hh1[s0:s0 + 16, :].T
            wh[X, r0 + 16:r0 + 32, r0 + 16:r0 + 32] = sc * W_hh2[s0:s0 + 16, :].T
    mk = np.ones((128, 1), np.float32)          # L1 mask: zero the L2 rows
    for g in range(G):
        mk[32 * g + 16:32 * g + 32] = 0.0
    return wq, wr, wh, wl, mk


def _build():
    import concourse.bacc as bacc
    import concourse.tile as tile
    from concourse import mybir
    from concourse.masks import make_identity
    f32, f32r, bf16 = mybir.dt.float32, mybir.dt.float32r, mybir.dt.bfloat16
    AF, ALU = mybir.ActivationFunctionType, mybir.AluOpType
    R = T
    nc = bacc.Bacc(None, target_bir_lowering=False)
    xt_d = nc.dram_tensor("xt", [T, 128, BG], bf16, kind="ExternalInput")
    wq_d = nc.dram_tensor("wq", [4, 128, 128], f32, kind="ExternalInput")
    wr_d = nc.dram_tensor("wr", [4, 128, 128], f32, kind="ExternalInput")
    wh_d = nc.dram_tensor("wh", [4, 128, 128], f32, kind="ExternalInput")
    wl_d = nc.dram_tensor("wl", [4, 128, 128], f32, kind="ExternalInput")
    mk_d = nc.dram_tensor("mk", [128, 1], f32, kind="ExternalInput")
    y_d = nc.dram_tensor("y", [B, H], f32, kind="ExternalOutput")

    with tile.TileContext(nc) as tc:
        with tc.tile_pool(name="singles", bufs=1) as singles, \
             tc.tile_pool(name="xt", bufs=2) as xtp, \
             tc.tile_pool(name="ys", bufs=4) as ysp, \
             tc.tile_pool(name="ps", bufs=4, space="PSUM") as psp:

            # ---- weights / constants. wq loads first: round 0 needs only
            # it (r1=h=0 there, so the wr/wh matmuls are skipped). ----
            def load_w(d, tag):
                wf = singles.tile([128, 4, 128], f32, tag=f"{tag}_f")
                nc.sync.dma_start(out=wf, in_=d.rearrange("x k m -> k x m"))
                w = singles.tile([128, 4, 128], bf16, tag=tag)
                nc.vector.tensor_copy(out=w, in_=wf)
                return w
            wq = load_w(wq_d, "wq")
            mk = singles.tile([128, 1], f32)
            nc.sync.dma_start(out=mk, in_=mk_d[:, :])
            ident = singles.tile([128, 128], f32)
            make_identity(nc, ident)

            # ---- states (per column-half) ----
            h_h, c_h, tg_h, r1_h, s_h, tp0_h, tp1_h, tn_h = [], [], [], [], [], [], [], []
            for i in range(NS):
                h_i = singles.tile([128, CW], bf16, tag=f"h{i}")
                nc.vector.memset(h_i, 0.0)
                h_h.append(h_i)
                c_i = singles.tile([128, CW], bf16, tag=f"c{i}")      # cell state
                nc.vector.memset(c_i, 0.0)
                c_h.append(c_i)
                tg_i = singles.tile([128, CW], bf16, tag=f"tg{i}")    # tanh(G)
                tg_h.append(tg_i)
                r1_i = singles.tile([128, CW], bf16, tag=f"r1{i}")    # relu(h)
                nc.vector.memset(r1_i, 0.0)
                r1_h.append(r1_i)
                s_i = singles.tile([128, 4, CW], bf16, tag=f"s{i}")   # sF sI sG2 sO
                s_h.append(s_i)
                tp0_i = singles.tile([128, CW], bf16, tag=f"tp0{i}")  # f*c
                tp0_h.append(tp0_i)
                tp1_i = singles.tile([128, CW], bf16, tag=f"tp1{i}")  # i*g
                tp1_h.append(tp1_i)
                tn_i = singles.tile([128, CW], bf16, tag=f"tn{i}")    # tanh(c)
                tn_h.append(tn_i)

            xts = {}

            def emit_xt(t):
                xt = xtp.tile([128, BG], bf16, tag=f"xt{t % 4}")
                nc.sync.dma_start(out=xt, in_=xt_d[t, :, :])
                xts[t] = xt

            emit_xt(0)
            emit_xt(1)
            emit_xt(2)
            next_xt = 3
            wr = load_w(wr_d, "wr")
            wh = load_w(wh_d, "wh")
            wl = load_w(wl_d, "wl")

            for r in range(R + 1):
                l1 = r < R
                if l1:
                    while next_xt <= min(r + 4, R - 1):
                        emit_xt(next_xt)
                        next_xt += 1
                    xt_cur, wsel = xts[r], wq
                else:
                    xt_cur, wsel = xts[R - 1], wl
                # PSUM planes: F=0, I=1, O=2, G=3. Phase-interleave the two
                # column halves so the in-order Act engine never head-of-line
                # blocks: tgA sigA tgB sigB tnA tnB.
                Ps = []
                for hf in range(NS):
                    cs = slice(CW * hf, CW * (hf + 1))
                    P = psp.tile([128, 4, CW], f32)
                    Ps.append(P)
                    # One open PSUM accumulation group per bank (planes
                    # {0,1}=F,I and {2,3}=G,O share banks). wq_F/wq_G have no
                    # state dep and pre-run while PE waits on r1; F and G
                    # chains close before I opens so sigmoid(F,I,G) fires at
                    # the 9th matmul.
                    def mm(X, w, rhs, st, sp):
                        nc.tensor.matmul(P[:, X, :], w[:, X, :], rhs,
                                         start=st, stop=sp, skip_group_check=True)
                    if r == 0:
                        # round 0: r1 = h = 0, so gates are x-proj + bias only
                        for X in (0, 1, 2, 3):
                            mm(X, wsel, xt_cur[:, cs], True, True)
                    else:
                        mm(0, wsel, xt_cur[:, cs], True, False)   # wq_F
                        mm(2, wsel, xt_cur[:, cs], True, False)   # wq_G
                        mm(0, wr, r1_h[hf], False, False)
                        mm(2, wr, r1_h[hf], False, False)
                        mm(0, wh, h_h[hf], False, True)           # F stop
                        mm(2, wh, h_h[hf], False, True)           # G stop
                        mm(1, wsel, xt_cur[:, cs], True, False)   # wq_I
                        mm(1, wr, r1_h[hf], False, False)
                        mm(1, wh, h_h[hf], False, True)           # I stop
                        mm(3, wsel, xt_cur[:, cs], True, False)   # wq_O
                        mm(3, wr, r1_h[hf], False, False)
                        mm(3, wh, h_h[hf], False, True)           # O stop
                for hf in range(NS):
                    # One fused sigmoid over F,I,G2 planes (g weights carry x2:
                    # tanh(x) = 2*sigmoid(2x)-1 reconstructed on DVE below).
                    nc.scalar.activation(out=s_h[hf][:, 0:3, :], in_=Ps[hf][:, 0:3, :],
                                         func=AF.Sigmoid)
                    nc.scalar.activation(out=s_h[hf][:, 3, :], in_=Ps[hf][:, 3, :],
                                         func=AF.Sigmoid)
                for hf in range(NS):
                    nc.vector.tensor_tensor(out=tp0_h[hf], in0=s_h[hf][:, 0, :],
                                            in1=c_h[hf], op=ALU.mult)
                    nc.vector.tensor_scalar(out=tg_h[hf], in0=s_h[hf][:, 2, :],
                                            scalar1=2.0, scalar2=-1.0,
                                            op0=ALU.mult, op1=ALU.add)
                    nc.vector.tensor_tensor(out=tp1_h[hf], in0=s_h[hf][:, 1, :],
                                            in1=tg_h[hf], op=ALU.mult)
                    nc.vector.tensor_tensor(out=c_h[hf], in0=tp0_h[hf],
                                            in1=tp1_h[hf], op=ALU.add)
                for hf in range(NS):
                    nc.scalar.activation(out=tn_h[hf], in_=c_h[hf], func=AF.Tanh)
                for hf in range(NS):
                    # r1 = relu(h) = max(tanh(c),0)*sig(O) since sig(O)>0.
                    nc.vector.scalar_tensor_tensor(
                        out=r1_h[hf], in0=tn_h[hf], scalar=0.0, in1=s_h[hf][:, 3, :],
                        op0=ALU.max, op1=ALU.mult)
                    if l1:
                        nc.gpsimd.tensor_tensor(out=h_h[hf], in0=s_h[hf][:, 3, :],
                                                in1=tn_h[hf], op=ALU.mult)
                if r == 0:
                    # L2 starts at r=1 from zero state: mask-wipe its rows
                    for hf in range(NS):
                        nc.vector.tensor_scalar(out=h_h[hf], in0=h_h[hf], scalar1=mk,
                                                scalar2=None, op0=ALU.mult)
                        nc.vector.tensor_scalar(out=c_h[hf], in0=c_h[hf],
                                                scalar1=mk, scalar2=None, op0=ALU.mult)

            # ---- output: r1 == relu(h); widen to f32, transpose out ----
            yrf = singles.tile([128, BG], f32)
            for hf in range(NS):
                nc.vector.tensor_copy(out=yrf[:, CW * hf:CW * (hf + 1)],
                                      in_=r1_h[hf])
            yv = y_d.rearrange("(g b) h -> b g h", g=G)  # [512, 4, 16]
            for j in range(4):
                P = psp.tile([128, 4, CW], f32)
                po = P[:, 0, 0:128]
                nc.tensor.transpose(po, yrf[:, 128 * j:128 * (j + 1)], ident)
                ys = ysp.tile([128, 128], f32, tag="ys")
                nc.vector.tensor_copy(out=ys, in_=po)
                nc.sync.dma_start(
                    out=yv[128 * j:128 * (j + 1), :, :],
                    in_=ys[:, :].rearrange("p (g w) -> p g w", w=32)[:, :, 16:32])
    nc.finalize()
    return nc


def _pack_x(x):
    """x [B*8, T, F] f32 -> per-core [T, 128, BG] bf16 tiles: row 32g+k holds
    feature k of batch-group g (k=18,19 are const 1.0 bias lanes)."""
    import ml_dtypes
    nb = x.shape[0] // B
    xp = np.zeros((nb, T, G, 32, BG), np.float32)
    xr = x.reshape(nb, G, BG, T, F)
    xp[:, :, :, 0:F, :] = xr.transpose(0, 3, 1, 4, 2)
    xp[:, :, :, F:F + 2, :] = 1.0
    return np.ascontiguousarray(
        xp.reshape(nb, T, 128, BG)).astype(ml_dtypes.bfloat16)


def kernel(x, W_ih1, W_hh1, b_ih1, b_hh1, W_ih2, W_hh2, b_ih2, b_hh2):
    from concourse.bass_utils import run_bass_kernel_spmd
    if "nc" not in _cache:
        _cache["nc"] = _build()
    nc = _cache["nc"]
    wq, wr, wh, wl, mk = _pack_weights(
        np.asarray(W_ih1), np.asarray(W_hh1), np.asarray(b_ih1), np.asarray(b_hh1),
        np.asarray(W_ih2), np.asarray(W_hh2), np.asarray(b_ih2), np.asarray(b_hh2))
    x = np.ascontiguousarray(np.asarray(x), dtype=np.float32)
    xp = _pack_x(x)
    in_maps = [dict(xt=xp[i], wq=wq, wr=wr, wh=wh, wl=wl, mk=mk)
               for i in range(N_CORES)]
    res = run_bass_kernel_spmd(nc, in_maps, core_ids=list(range(N_CORES)))
    return np.concatenate([res.results[i]["y"] for i in range(N_CORES)], axis=0)


# revision 7
# speedup vs baseline: 1.0269x; 1.0026x over previous
"""TRN2 Bass kernel for nn_Encoder: 2-layer LSTM encoder, batch-sharded 8 ways.

Per core (B=2048): 4 batch groups x 512 batch columns; partition row =
32g + 16*layer + j; L2 runs one timestep behind L1 in the same tiles, so one
set of matmuls serves both layers. Columns split in 2 halves (CW=256) that
pipeline against each other.

Per round and half, each gate plane is 3 bf16 matmuls accumulated in PSUM:
block-diagonal x-projection (lhsT has per-group W_ih1 blocks + bias rows fed
by constant-1 lanes of x), wr (relu(h1) -> L2 inputs), wh (recurrence); PSUM
allows one open accumulation group per bank, so plane chains close before a
same-bank plane starts, with the two x-projections of bank-distinct planes
pre-issued while PE waits on the recurrence.

Elementwise: the g-gate weights carry x2 so one fused sigmoid covers F/I/G
planes (tanh(x) = 2*sigmoid(2x)-1, reconstructed by a 4x-mode tensor_scalar);
sigma(O) separate; cell state kept in bf16; relu(h) computed directly as
max(tanh(c),0)*sigma(O) via scalar_tensor_tensor; h on GPSIMD. Host-side
packing pre-transposes x into per-timestep [128, 512] bf16 tiles (one plain
DMA per round) and packs the weight/bias matrices.
"""
import sys
sys.path.insert(0, '/opt/trn_rl_repo')
import numpy as np

B, T, F, H = 2048, 100, 18, 16
G = 4
BG = B // G          # 512 batch per group
NS = 2               # column halves
CW = BG // NS        # 256
N_CORES = 8
SLAB = 10            # timesteps per staging slab
NSLAB = T // SLAB
PLANE_SLICE = [16, 0, 32, 48]   # PSUM plane X -> weight row block (f,i,g,o)
PLANE_SCALE = [1.0, 1.0, 2.0, 1.0]  # g-plane x2: tanh(x) = 2*sigmoid(2x)-1

_cache = {}


def _pack_weights(W_ih1, W_hh1, b_ih1, b_hh1, W_ih2, W_hh2, b_ih2, b_hh2):
    import ml_dtypes
    wq = np.zeros((4, 128, 128), np.float32)   # block-diag x-proj lhsT (-> bf16)
    wl = np.zeros((4, 128, 128), np.float32)   # bias-only lhsT for final round
    wr = np.zeros((4, 128, 128), np.float32)   # L2-input proj from relu(h)
    wh = np.zeros((4, 128, 128), np.float32)   # recurrence
    for X in range(4):
        s0 = PLANE_SLICE[X]
        sc = PLANE_SCALE[X]
        b1 = sc * (b_ih1[s0:s0 + 16] + b_hh1[s0:s0 + 16])
        b2 = sc * (b_ih2[s0:s0 + 16] + b_hh2[s0:s0 + 16])
        b1a = b1.astype(ml_dtypes.bfloat16).astype(np.float32)
        b2a = b2.astype(ml_dtypes.bfloat16).astype(np.float32)
        for g in range(G):
            r0 = 32 * g
            wq[X, r0:r0 + F, r0:r0 + 16] = sc * W_ih1[s0:s0 + 16, :].T
            wq[X, r0 + F, r0:r0 + 16] = b1a
            wq[X, r0 + F, r0 + 16:r0 + 32] = b2a
            wq[X, r0 + F + 1, r0:r0 + 16] = b1 - b1a
            wq[X, r0 + F + 1, r0 + 16:r0 + 32] = b2 - b2a
            wl[X, r0 + F, r0:r0 + 16] = b1a
            wl[X, r0 + F, r0 + 16:r0 + 32] = b2a
            wl[X, r0 + F + 1, r0:r0 + 16] = b1 - b1a
            wl[X, r0 + F + 1, r0 + 16:r0 + 32] = b2 - b2a
            wr[X, r0:r0 + 16, r0 + 16:r0 + 32] = sc * W_ih2[s0:s0 + 16, :].T
            wh[X, r0:r0 + 16, r0:r0 + 16] = sc * W_hh1[s0:s0 + 16, :].T
            wh[X, r0 + 16:r0 + 32, r0 + 16:r0 + 32] = sc * W_hh2[s0:s0 + 16, :].T
    mk = np.ones((128, 1), np.float32)          # L1 mask: zero the L2 rows
    for g in range(G):
        mk[32 * g + 16:32 * g + 32] = 0.0
    return wq, wr, wh, wl, mk


def _build():
    import concourse.bacc as bacc
    import concourse.tile as tile
    from concourse import mybir
    from concourse.masks import make_identity
    f32, f32r, bf16 = mybir.dt.float32, mybir.dt.float32r, mybir.dt.bfloat16
    AF, ALU = mybir.ActivationFunctionType, mybir.AluOpType
    R = T
    nc = bacc.Bacc(None, target_bir_lowering=False)
    xt_d = nc.dram_tensor("xt", [T, 128, BG], bf16, kind="ExternalInput")
    wq_d = nc.dram_tensor("wq", [4, 128, 128], f32, kind="ExternalInput")
    wr_d = nc.dram_tensor("wr", [4, 128, 128], f32, kind="ExternalInput")
    wh_d = nc.dram_tensor("wh", [4, 128, 128], f32, kind="ExternalInput")
    wl_d = nc.dram_tensor("wl", [4, 128, 128], f32, kind="ExternalInput")
    mk_d = nc.dram_tensor("mk", [128, 1], f32, kind="ExternalInput")
    y_d = nc.dram_tensor("y", [B, H], f32, kind="ExternalOutput")

    with tile.TileContext(nc) as tc:
        with tc.tile_pool(name="singles", bufs=1) as singles, \
             tc.tile_pool(name="xt", bufs=2) as xtp, \
             tc.tile_pool(name="ys", bufs=4) as ysp, \
             tc.tile_pool(name="ps", bufs=4, space="PSUM") as psp:

            # ---- weights / constants. wq loads first: round 0 needs only
            # it (r1=h=0 there, so the wr/wh matmuls are skipped). ----
            def load_w(d, tag):
                wf = singles.tile([128, 4, 128], f32, tag=f"{tag}_f")
                nc.sync.dma_start(out=wf, in_=d.rearrange("x k m -> k x m"))
                w = singles.tile([128, 4, 128], bf16, tag=tag)
                nc.vector.tensor_copy(out=w, in_=wf)
                return w
            wq = load_w(wq_d, "wq")

            # ---- states (per column-half) ----
            h_h, c_h, tg_h, r1_h, s_h, tp0_h, tp1_h, tn_h = [], [], [], [], [], [], [], []
            for i in range(NS):
                h_i = singles.tile([128, CW], bf16, tag=f"h{i}")
                nc.vector.memset(h_i, 0.0)
                h_h.append(h_i)
                c_i = singles.tile([128, CW], bf16, tag=f"c{i}")      # cell state
                nc.vector.memset(c_i, 0.0)
                c_h.append(c_i)
                tg_i = singles.tile([128, CW], bf16, tag=f"tg{i}")    # tanh(G)
                tg_h.append(tg_i)
                r1_i = singles.tile([128, CW], bf16, tag=f"r1{i}")    # relu(h)
                nc.vector.memset(r1_i, 0.0)
                r1_h.append(r1_i)
                s_i = singles.tile([128, 4, CW], bf16, tag=f"s{i}")   # sF sI sG2 sO
                s_h.append(s_i)
                tp0_i = singles.tile([128, CW], bf16, tag=f"tp0{i}")  # f*c
                tp0_h.append(tp0_i)
                tp1_i = singles.tile([128, CW], bf16, tag=f"tp1{i}")  # i*g
                tp1_h.append(tp1_i)
                tn_i = singles.tile([128, CW], bf16, tag=f"tn{i}")    # tanh(c)
                tn_h.append(tn_i)

            xts = {}

            def emit_xt(t):
                xt = xtp.tile([128, BG], bf16, tag=f"xt{t % 4}")
                nc.sync.dma_start(out=xt, in_=xt_d[t, :, :])
                xts[t] = xt

            emit_xt(0)
            emit_xt(1)
            emit_xt(2)
            next_xt = 3
            wr = load_w(wr_d, "wr")
            wh = load_w(wh_d, "wh")
            wl = load_w(wl_d, "wl")
            mk = singles.tile([128, 1], f32)
            nc.sync.dma_start(out=mk, in_=mk_d[:, :])

            for r in range(R + 1):
                l1 = r < R
                if l1:
                    while next_xt <= min(r + 4, R - 1):
                        emit_xt(next_xt)
                        next_xt += 1
                    xt_cur, wsel = xts[r], wq
                else:
                    xt_cur, wsel = xts[R - 1], wl
                # PSUM planes: F=0, I=1, O=2, G=3. Phase-interleave the two
                # column halves so the in-order Act engine never head-of-line
                # blocks: tgA sigA tgB sigB tnA tnB.
                Ps = []
                for hf in range(NS):
                    cs = slice(CW * hf, CW * (hf + 1))
                    P = psp.tile([128, 4, CW], f32)
                    Ps.append(P)
                    # One open PSUM accumulation group per bank (planes
                    # {0,1}=F,I and {2,3}=G,O share banks). wq_F/wq_G have no
                    # state dep and pre-run while PE waits on r1; F and G
                    # chains close before I opens so sigmoid(F,I,G) fires at
                    # the 9th matmul.
                    def mm(X, w, rhs, st, sp):
                        nc.tensor.matmul(P[:, X, :], w[:, X, :], rhs,
                                         start=st, stop=sp, skip_group_check=True)
                    if r == 0:
                        # round 0: r1 = h = 0, so gates are x-proj + bias only
                        for X in (0, 1, 2, 3):
                            mm(X, wsel, xt_cur[:, cs], True, True)
                    else:
                        mm(0, wsel, xt_cur[:, cs], True, False)   # wq_F
                        mm(2, wsel, xt_cur[:, cs], True, False)   # wq_G
                        mm(0, wr, r1_h[hf], False, False)
                        mm(2, wr, r1_h[hf], False, False)
                        mm(0, wh, h_h[hf], False, True)           # F stop
                        mm(2, wh, h_h[hf], False, True)           # G stop
                        mm(1, wsel, xt_cur[:, cs], True, False)   # wq_I
                        mm(1, wr, r1_h[hf], False, False)
                        mm(1, wh, h_h[hf], False, True)           # I stop
                        mm(3, wsel, xt_cur[:, cs], True, False)   # wq_O
                        mm(3, wr, r1_h[hf], False, False)
                        mm(3, wh, h_h[hf], False, True)           # O stop
                for hf in range(NS):
                    # One fused sigmoid over F,I,G2 planes (g weights carry x2:
                    # tanh(x) = 2*sigmoid(2x)-1 reconstructed on DVE below).
                    nc.scalar.activation(out=s_h[hf][:, 0:3, :], in_=Ps[hf][:, 0:3, :],
                                         func=AF.Sigmoid)
                    nc.scalar.activation(out=s_h[hf][:, 3, :], in_=Ps[hf][:, 3, :],
                                         func=AF.Sigmoid)
                for hf in range(NS):
                    nc.vector.tensor_tensor(out=tp0_h[hf], in0=s_h[hf][:, 0, :],
                                            in1=c_h[hf], op=ALU.mult)
                    nc.vector.tensor_scalar(out=tg_h[hf], in0=s_h[hf][:, 2, :],
                                            scalar1=2.0, scalar2=-1.0,
                                            op0=ALU.mult, op1=ALU.add)
                    nc.vector.tensor_tensor(out=tp1_h[hf], in0=s_h[hf][:, 1, :],
                                            in1=tg_h[hf], op=ALU.mult)
                    nc.vector.tensor_tensor(out=c_h[hf], in0=tp0_h[hf],
                                            in1=tp1_h[hf], op=ALU.add)
                for hf in range(NS):
                    nc.scalar.activation(out=tn_h[hf], in_=c_h[hf], func=AF.Tanh)
                for hf in range(NS):
                    # r1 = relu(h) = max(tanh(c),0)*sig(O) since sig(O)>0.
                    nc.vector.scalar_tensor_tensor(
                        out=r1_h[hf], in0=tn_h[hf], scalar=0.0, in1=s_h[hf][:, 3, :],
                        op0=ALU.max, op1=ALU.mult)
                    if l1:
                        nc.gpsimd.tensor_tensor(out=h_h[hf], in0=s_h[hf][:, 3, :],
                                                in1=tn_h[hf], op=ALU.mult)
                if r == 0:
                    # L2 starts at r=1 from zero state: mask-wipe its rows
                    for hf in range(NS):
                        nc.vector.tensor_scalar(out=h_h[hf], in0=h_h[hf], scalar1=mk,
                                                scalar2=None, op0=ALU.mult)
                        nc.vector.tensor_scalar(out=c_h[hf], in0=c_h[hf],
                                                scalar1=mk, scalar2=None, op0=ALU.mult)

            # ---- output: r1 == relu(h); widen to f32, transpose out ----
            ident = singles.tile([128, 128], f32)
            make_identity(nc, ident)
            yrf = singles.tile([128, BG], f32)
            for hf in range(NS):
                nc.vector.tensor_copy(out=yrf[:, CW * hf:CW * (hf + 1)],
                                      in_=r1_h[hf])
            yv = y_d.rearrange("(g b) h -> b g h", g=G)  # [512, 4, 16]
            for j in range(4):
                P = psp.tile([128, 4, CW], f32)
                po = P[:, 0, 0:128]
                nc.tensor.transpose(po, yrf[:, 128 * j:128 * (j + 1)], ident)
                ys = ysp.tile([128, 128], f32, tag="ys")
                nc.vector.tensor_copy(out=ys, in_=po)
                nc.sync.dma_start(
                    out=yv[128 * j:128 * (j + 1), :, :],
                    in_=ys[:, :].rearrange("p (g w) -> p g w", w=32)[:, :, 16:32])
    nc.finalize()
    return nc


def _pack_x(x):
    """x [B*8, T, F] f32 -> per-core [T, 128, BG] bf16 tiles: row 32g+k holds
    feature k of batch-group g (k=18,19 are const 1.0 bias lanes)."""
    import ml_dtypes
    nb = x.shape[0] // B
    xp = np.zeros((nb, T, G, 32, BG), np.float32)
    xr = x.reshape(nb, G, BG, T, F)
    xp[:, :, :, 0:F, :] = xr.transpose(0, 3, 1, 4, 2)
    xp[:, :, :, F:F + 2, :] = 1.0
    return np.ascontiguousarray(
        xp.reshape(nb, T, 128, BG)).astype(ml_dtypes.bfloat16)


def kernel(x, W_ih1, W_hh1, b_ih1, b_hh1, W_ih2, W_hh2, b_ih2, b_hh2):
    from concourse.bass_utils import run_bass_kernel_spmd
    if "nc" not in _cache:
        _cache["nc"] = _build()
    nc = _cache["nc"]
    wq, wr, wh, wl, mk = _pack_weights(
        np.asarray(W_ih1), np.asarray(W_hh1), np.asarray(b_ih1), np.asarray(b_hh1),
        np.asarray(W_ih2), np.asarray(W_hh2), np.asarray(b_ih2), np.asarray(b_hh2))
    x = np.ascontiguousarray(np.asarray(x), dtype=np.float32)
    xp = _pack_x(x)
    in_maps = [dict(xt=xp[i], wq=wq, wr=wr, wh=wh, wl=wl, mk=mk)
               for i in range(N_CORES)]
    res = run_bass_kernel_spmd(nc, in_maps, core_ids=list(range(N_CORES)))
    return np.concatenate([res.results[i]["y"] for i in range(N_CORES)], axis=0)
